# revision 1
# baseline (speedup 1.0000x reference)
"""Masked multi-head attention on 8 Trainium2 NeuronCores.

Sharding: core c = (b, hg) with b = c // 4, hg = c % 4. Each core computes the
full attention block for batch b restricted to heads [4*hg, 4*hg+4), including
its slice of the QKV projection and of the output projection. The host sums the
4 tensor-parallel partial outputs per batch and adds the output bias.

Shapes are hardcoded for B=2, T=2048, D=1024, H=16, Hd=64, fp32.
"""

import numpy as np
from contextlib import ExitStack

import concourse.bass as bass
import concourse.bacc as bacc
import concourse.mybir as mybir
import concourse.tile as tile
from concourse.bass_utils import run_bass_kernel_spmd

B, T, D = 2, 2048, 1024
H, HD = 16, 64
HL = 4               # heads per core
NCORES = 8
TQ = 512             # query tile (matmul moving free dim)
TK = 128             # key tile
NQT = T // TQ        # 4
NKT = T // TK        # 16
NDT = D // 128       # 8

F32 = mybir.dt.float32
F32R = mybir.dt.float32r
EXP = mybir.ActivationFunctionType.Exp
MULT = mybir.AluOpType.mult

LAST_RESULTS = None  # BassKernelResults of the most recent run (for test.py)


def r(ap):
    return ap if ap.dtype == F32R else ap.bitcast(F32R)


def _build_mha(tc, out_ap, in_aps):
    nc = tc.nc
    x_d = in_aps["x"]          # [T, D]
    wqk_d = in_aps["wqk"]      # [D, 512]  (Qh0|Qh1|Qh2|Qh3|Kh0..Kh3, Q pre-scaled)
    bqk_d = in_aps["bqk"]      # [1, 512]
    wv_d = in_aps["wv"]        # [D, 256]
    bv_d = in_aps["bv"]        # [1, 256]
    wout_d = in_aps["wout"]    # [128, 2, 1024]
    mask_d = in_aps["mask"]    # [128, 896]
    ones_d = in_aps["ones"]    # [128, 512] all-ones

    with ExitStack() as ctx:
        ctx.enter_context(nc.allow_low_precision(reason="fp32r matmul pipeline"))
        const = ctx.enter_context(tc.tile_pool(name="const", bufs=1))
        big = ctx.enter_context(tc.tile_pool(name="big", bufs=1))

        ident = const.tile([128, 128], F32)
        from concourse.masks import make_identity
        make_identity(nc, ident[:])

        # Persistent activations.
        # qkt[:, fb, t]: fb 0,1 = Q^T head pairs (0,1),(2,3); fb 2,3 = K^T pairs.
        # Rows 0:64 = even head of the pair, 64:128 = odd head.
        qkt = big.tile([128, 4, T], F32R)
        # vp[:, kt, h, 0:64] = V[kt*128:+128, h*64:+64]; vp[..., 64] = 1.0
        vp = big.tile([128, NKT, HL, 65], F32R)
        # ot[:, p, t]: normalized attention output^T; rows by head as in qkt
        ot = big.tile([128, 2, T], F32R)

        # ---- Interleaved pipeline over 512-token blocks ----
        # One shared PSUM pool; tags sized so all concurrent users fit in the
        # 8 banks: pt(2) + pq(2) + pv(2) + av0(1) + av1(1).
        with ExitStack() as pctx:
            ps = pctx.enter_context(tc.tile_pool(name="ps", bufs=2, space="PSUM"))
            ps_av = pctx.enter_context(tc.tile_pool(name="ps_av", bufs=1,
                                                    space="PSUM"))
            wpool = pctx.enter_context(tc.tile_pool(name="w", bufs=1))
            xin_p = pctx.enter_context(tc.tile_pool(name="xin", bufs=5))
            xt_p = pctx.enter_context(tc.tile_pool(name="xt", bufs=2))
            pt_p = pctx.enter_context(tc.tile_pool(name="ptile", bufs=6))
            nrm_p = pctx.enter_context(tc.tile_pool(name="nrm", bufs=4))
            ob_p = pctx.enter_context(tc.tile_pool(name="ob", bufs=4))

            # x tiles for the first token block go out first so the PE can
            # start transposing immediately; weights/consts queue behind them
            pre = []
            for ts in range(4):
                xin = xin_p.tile([128, D], F32, tag="xin", name=f"xin_p{ts}")
                nc.sync.dma_start(
                    xin[:], x_d[ts * 128:(ts + 1) * 128, :])
                pre.append(xin)
            wqk = wpool.tile([128, NDT, 512], F32R)
            nc.sync.dma_start(wqk[:], wqk_d.rearrange("(o p) f -> p o f", p=128).bitcast(F32R))
            wv = wpool.tile([128, NDT, 256], F32R)
            nc.sync.dma_start(wv[:], wv_d.rearrange("(o p) f -> p o f", p=128).bitcast(F32R))
            ones = const.tile([1, 512], F32R)
            nc.sync.dma_start(ones[:], ones_d[0:1, :].bitcast(F32R))
            bqk = const.tile([1, 512], F32R)
            nc.sync.dma_start(bqk[:], bqk_d.bitcast(F32R))
            bv = const.tile([1, 256], F32R)
            nc.sync.dma_start(bv[:], bv_d.bitcast(F32R))
            mask = const.tile([128, 896], F32R)
            nc.sync.dma_start(mask[:], mask_d.bitcast(F32R))
            nc.sync.dma_start(
                vp[:, :, :, 64],
                ones_d[:, 0:NKT * HL].rearrange("p (a b) -> p a b", b=HL).bitcast(F32R))
            wout = const.tile([128, 2, 1024], F32R)
            nc.sync.dma_start(wout[:], wout_d.bitcast(F32R))

            def emit_transpose_group(tt, ts, dh, xt):
                """4 PE transposes into one PSUM bank + 1 DVE evacuation."""
                if tt == 0:
                    xin = pre[ts]
                else:
                    xin = xins[(tt, ts)]
                pt = ps.tile([128, 512], F32, tag="pt",
                             name=f"tp_{tt}_{ts}_{dh}")
                for dj in range(4):
                    dt = dh * 4 + dj
                    nc.tensor.transpose(
                        pt[:, dj * 128:(dj + 1) * 128],
                        xin[:, dt * 128:(dt + 1) * 128], ident[:])
                nc.vector.tensor_copy(
                    xt[:, dh * 4:(dh + 1) * 4, ts * 128:(ts + 1) * 128],
                    pt[:].rearrange("p (a b) -> p a b", b=128))

            def emit_x_dma(tt):
                if tt == 0:
                    return
                for ts in range(4):
                    xin = xin_p.tile([128, D], F32, tag="xin",
                                     name=f"xin_{tt}_{ts}")
                    nc.sync.dma_start(
                        xin[:],
                        x_d[tt * TQ + ts * 128 : tt * TQ + (ts + 1) * 128, :])
                    xins[(tt, ts)] = xin

            def transpose_fillers(tt):
                xt = xt_p.tile([128, NDT, TQ], F32R, tag="xt", name=f"xt_{tt}")
                xts[tt] = xt
                return [
                    (lambda tt=tt, ts=ts, dh=dh, xt=xt:
                     emit_transpose_group(tt, ts, dh, xt))
                    for ts in range(4) for dh in range(2)
                ]

            def emit_fb(tt, fb):
                xt = xts[tt]
                pq = ps.tile([128, TQ], F32, tag="pq", name=f"pq_{tt}_{fb}")
                for dt in range(NDT):
                    nc.tensor.matmul(pq[:], r(wqk[:, dt, fb * 128:(fb + 1) * 128]),
                                     r(xt[:, dt, :]),
                                     start=(dt == 0), stop=False)
                nc.tensor.matmul(pq[:], r(bqk[0:1, fb * 128:(fb + 1) * 128]),
                                 r(ones[0:1, :]), start=False, stop=True)
                nc.vector.tensor_copy(qkt[:, fb, tt * TQ:(tt + 1) * TQ], pq[:])

            def emit_v(tt, ts):
                xt = xts[tt]
                pv = ps.tile([128, 512], F32, tag="pv", name=f"pv_{tt}_{ts}")
                for dt in range(NDT):
                    nc.tensor.matmul(pv[:, 0:256],
                                     r(xt[:, dt, ts * 128:(ts + 1) * 128]),
                                     r(wv[:, dt, :]), start=(dt == 0), stop=False)
                nc.tensor.matmul(pv[:, 0:256], r(ones[0:1, 0:128]), r(bv[0:1, :]),
                                 start=False, stop=True)
                nc.vector.tensor_copy(
                    vp[:, tt * 4 + ts, :, 0:64],
                    pv[:, 0:256].rearrange("p (h e) -> p h e", e=HD))

            def emit_b_qkv(tt):
                """Q^T/K^T columns + V rows from the prepared x^T block."""
                for fb in range(4):
                    emit_fb(tt, fb)
                for ts in range(4):
                    emit_v(tt, ts)

            def emit_scores(p, qi, kt):
                """QK^T + exp (+ causal mask on diagonal tiles) -> P^T tiles.

                Diagonal tiles (rr = kt-4qi in 0..3) only need columns
                >= 128*rr; compute cols [c_lo, TQ) with c_lo capped at 256 so
                the fp32r moving dim stays >= 256, and mask-multiply only the
                column range that contains zeros.
                """
                rr = kt - 4 * qi
                c_lo = 0 if rr < 0 else min(128 * rr, 256)
                pts = []
                for a in range(2):          # head within pair
                    rows = slice(64 * a, 64 * a + 64)
                    s = ps.tile([128, TQ], F32, tag=("pq" if a == 0 else "pt"),
                                name=f"s{a}_{p}_{qi}_{kt}")
                    nc.tensor.matmul(
                        s[:, c_lo:], r(qkt[rows, 2 + p, kt * TK:(kt + 1) * TK]),
                        r(qkt[rows, p, qi * TQ + c_lo:(qi + 1) * TQ]),
                        start=True, stop=True)
                    pt = pt_p.tile([128, TQ], F32R, tag=f"pt{a}",
                                   name=f"pt{a}_{p}_{qi}_{kt}")
                    nc.scalar.activation(pt[:, c_lo:], s[:, c_lo:], EXP)
                    if rr >= 0:
                        c0 = (3 - rr) * 128
                        m_lo, m_hi = c_lo, min(128 * rr + 128, TQ)
                        nc.vector.tensor_tensor(
                            pt[:, m_lo:m_hi], pt[:, m_lo:m_hi],
                            mask[:, c0 + m_lo:c0 + m_hi], MULT)
                    pts.append(pt)
                return pts, c_lo

            def emit_c(p, qi, fillers):
                av = [ps_av.tile([128, TQ], F32, tag=f"av{a}",
                                 name=f"av{a}_{p}_{qi}") for a in range(2)]
                nkt = 4 * qi + 4            # causal: k tiles 0 .. 4qi+3
                pts, c_lo = emit_scores(p, qi, 0)
                for kt in range(nkt):
                    # next kt's scores go ahead of this kt's AV, and one unit
                    # of independent PE work (transpose group / out-proj) is
                    # slotted in so the PE isn't gated on the current exp
                    nxt = emit_scores(p, qi, kt + 1) if kt + 1 < nkt else (None, 0)
                    n_pop = min(len(fillers), max(1, -(-len(fillers) // (nkt - kt))))
                    for _ in range(n_pop):
                        fillers.popleft()()
                    for a in range(2):
                        nc.tensor.matmul(
                            av[a][0:65, c_lo:], r(vp[:, kt, 2 * p + a, :]),
                            r(pts[a][:, c_lo:]),
                            start=(kt == 0), stop=(kt == nkt - 1),
                            skip_group_check=True)
                    pts, c_lo = nxt
                # normalize: rows 0:64 are O^T, row 64 is the softmax denom
                for a in range(2):
                    rec = nrm_p.tile([1, TQ], F32R, tag="rec",
                                     name=f"rec_{p}_{qi}_{a}")
                    nc.vector.reciprocal(rec[:], av[a][64:65, :])
                    pb = ps.tile([64, TQ], F32, tag="pt", name=f"pb_{p}_{qi}_{a}")
                    nc.tensor.matmul(pb[:], r(ones[0:1, 0:64]), r(rec[:]),
                                     start=True, stop=True)
                    bc = nrm_p.tile([64, TQ], F32, tag="bc",
                                    name=f"bc_{p}_{qi}_{a}")
                    nc.vector.tensor_copy(bc[:], pb[:])
                    nc.vector.tensor_tensor(
                        ot[64 * a:64 * a + 64, p, qi * TQ:(qi + 1) * TQ],
                        av[a][0:64, :], bc[:], MULT)

            def po_fillers(qi):
                def emit_po(ts, dt):
                    po = ps.tile([128, 512], F32, tag="pv",
                                 name=f"po_{ts}_{dt}")
                    for ft in range(2):
                        nc.tensor.matmul(
                            po[:], r(ot[:, ft, ts * 128:(ts + 1) * 128]),
                            r(wout[:, ft, dt * 512:(dt + 1) * 512]),
                            start=(ft == 0), stop=(ft == 1))
                    ob = ob_p.tile([128, 512], F32, tag="ob",
                                   name=f"ob_{ts}_{dt}")
                    nc.vector.tensor_copy(ob[:], po[:])
                    nc.sync.dma_start(
                        out_ap[ts * 128:(ts + 1) * 128,
                               dt * 512:(dt + 1) * 512],
                        ob[:])
                return [
                    (lambda ts=ts, dt=dt: emit_po(ts, dt))
                    for ts in range(4 * qi, 4 * qi + 4) for dt in range(2)
                ]

            from collections import deque
            xins, xts = {}, {}
            for f in transpose_fillers(0):  # block 0's x^T up front
                f()
            last = NQT - 1
            for tt in range(NQT):
                if tt < last:
                    emit_b_qkv(tt)
                    fl = deque()
                    emit_x_dma(tt + 1)
                    tps = transpose_fillers(tt + 1)
                    pos = po_fillers(tt - 1) if tt >= 1 else []
                    # alternate so out-proj units land after their normalize
                    # producers have drained, while transposes still finish
                    # before the next block's QKV needs x^T
                    while tps or pos:
                        if tps:
                            fl.append(tps.pop(0))
                        if pos:
                            fl.append(pos.pop(0))
                    emit_c(0, tt, fl)
                    emit_c(1, tt, fl)
                    while fl:
                        fl.popleft()()
                else:
                    # last block: only pair 0's Q/K columns are needed up
                    # front; V, pair 1's columns, and D(tt-1) feed the
                    # C(p0) iteration slots
                    emit_fb(tt, 0)
                    emit_fb(tt, 2)
                    fl = deque()
                    fl.extend([(lambda ts=ts: emit_v(tt, ts))
                               for ts in range(4)])
                    fl.append(lambda: emit_fb(tt, 1))
                    fl.append(lambda: emit_fb(tt, 3))
                    fl.extend(po_fillers(tt - 1))
                    emit_c(0, tt, fl)
                    emit_c(1, tt, fl)
                    while fl:
                        fl.popleft()()
            for f in po_fillers(NQT - 1):
                f()


_CACHE = {}


def _program():
    if "nc" in _CACHE:
        return _CACHE["nc"]
    nc = bacc.Bacc("TRN2", target_bir_lowering=False, debug=False)
    ins = {
        "x": nc.dram_tensor("x", [T, D], F32, kind="ExternalInput").ap(),
        "wqk": nc.dram_tensor("wqk", [D, 512], F32, kind="ExternalInput").ap(),
        "bqk": nc.dram_tensor("bqk", [1, 512], F32, kind="ExternalInput").ap(),
        "wv": nc.dram_tensor("wv", [D, 256], F32, kind="ExternalInput").ap(),
        "bv": nc.dram_tensor("bv", [1, 256], F32, kind="ExternalInput").ap(),
        "wout": nc.dram_tensor("wout", [128, 2, 1024], F32,
                               kind="ExternalInput").ap(),
        "mask": nc.dram_tensor("mask", [128, 896], F32, kind="ExternalInput").ap(),
        "ones": nc.dram_tensor("ones", [128, 512], F32, kind="ExternalInput").ap(),
    }
    out = nc.dram_tensor("out", [T, D], F32, kind="ExternalOutput").ap()
    with tile.TileContext(nc) as tc:
        _build_mha(tc, out, ins)
    nc.compile()
    _CACHE["nc"] = nc
    return nc


def _in_maps(x, Wqkv, bqkv, Wout):
    x = np.asarray(x, dtype=np.float32)
    Wqkv = np.asarray(Wqkv, dtype=np.float32)
    bqkv = np.asarray(bqkv, dtype=np.float32)
    Wout = np.asarray(Wout, dtype=np.float32)
    scale = np.float32(1.0 / np.sqrt(HD))
    mask = (np.arange(128)[:, None] <= np.arange(896)[None, :] - 384).astype(
        np.float32)
    maps = []
    for c in range(NCORES):
        b, hg = c // 4, c % 4
        hs = [4 * hg + i for i in range(HL)]
        q_cols = np.concatenate([Wqkv[:, h * HD:(h + 1) * HD] for h in hs], axis=1)
        k_cols = np.concatenate(
            [Wqkv[:, D + h * HD:D + (h + 1) * HD] for h in hs], axis=1)
        v_cols = np.concatenate(
            [Wqkv[:, 2 * D + h * HD:2 * D + (h + 1) * HD] for h in hs], axis=1)
        bq = np.concatenate([bqkv[h * HD:(h + 1) * HD] for h in hs])
        bk = np.concatenate([bqkv[D + h * HD:D + (h + 1) * HD] for h in hs])
        bv_ = np.concatenate([bqkv[2 * D + h * HD:2 * D + (h + 1) * HD] for h in hs])
        wqk = np.ascontiguousarray(
            np.concatenate([q_cols * scale, k_cols], axis=1))
        bqk = np.concatenate([bq * scale, bk])[None, :]
        wo = np.concatenate([Wout[h * HD:(h + 1) * HD, :] for h in hs], axis=0)
        wo = np.ascontiguousarray(
            wo.reshape(2, 128, D).transpose(1, 0, 2))
        maps.append({
            "x": np.ascontiguousarray(x[b]),
            "wqk": wqk,
            "bqk": np.ascontiguousarray(bqk),
            "wv": np.ascontiguousarray(v_cols),
            "bv": np.ascontiguousarray(bv_[None, :]),
            "wout": wo,
            "mask": mask,
            "ones": np.ones((128, 512), dtype=np.float32),
        })
    return maps


def kernel(x, Wqkv, bqkv, Wout, bout):
    global LAST_RESULTS
    nc = _program()
    maps = _in_maps(x, Wqkv, bqkv, Wout)
    res = run_bass_kernel_spmd(nc, maps, list(range(NCORES)))
    LAST_RESULTS = res
    bout = np.asarray(bout, dtype=np.float32)
    out = np.empty((B, T, D), dtype=np.float32)
    for b in range(B):
        acc = res.results[4 * b]["out"].astype(np.float32)
        for hg in range(1, 4):
            acc = acc + res.results[4 * b + hg]["out"]
        out[b] = acc + bout[None, :]
    return out



# revision 6
# speedup vs baseline: 1.1007x; 1.1007x over previous
"""Masked multi-head attention on 8 Trainium2 NeuronCores.

Sharding: core c = (b, hg) with b = c // 4, hg = c % 4. Each core computes the
full attention block for batch b restricted to heads [4*hg, 4*hg+4), including
its slice of the QKV projection and of the output projection. The host sums the
4 tensor-parallel partial outputs per batch and adds the (V-bias-folded) output
bias.

Numerics: QKV / AV / out-projection run in bf16 (PE rate 1.0 cycles/row, same
as fp32r, but half the DMA/SBUF), the QK^T score matmuls run in fp8e4m3 with
DoubleRow perf mode (0.5 cycles/row). K-bias is dropped (softmax row-shift
invariance), V-bias is folded into the output bias on the host, Q-bias is
applied during PSUM evacuation.

Layouts (per core):
  xt   [128, 8, 2048]  bf16  x^T: xt[p, dt, t] = x[t, 128*dt + p]
  wqk  [128, 8, 512]   bf16  col blocks QA|QB|KA|KB; block col 32*i + j is
                             head (4*hg + i), dims j / 32+j (A/B half).
                             Q pre-scaled by 1/sqrt(Hd).
  qkt  [128, 2, 2, T]  fp8   [32h+j, half, q/k, t] per-head Q^T/K^T
  wv   [128, 8, 256]   bf16
  vp   [128, 4, 16, 65] bf16 V tiles per (head, k-tile); col 64 = ones row
                             (softmax denominator trick)
  ot   [128, 2, T]     bf16  normalized O^T; rows 64*(h%2) + d at dim1 h//2
  wout [128, 2, 1024]  bf16
Shapes hardcoded for B=2, T=2048, D=1024, H=16, Hd=64.
"""

import numpy as np
import ml_dtypes
from collections import deque
from contextlib import ExitStack

import concourse.bass as bass
import concourse.bacc as bacc
import concourse.mybir as mybir
import concourse.tile as tile
from concourse.bass_utils import run_bass_kernel_spmd

B, T, D = 2, 2048, 1024
H, HD = 16, 64
HL = 4               # heads per core
NCORES = 8
TQ = 512             # query tile
TK = 128             # key tile
NQT = T // TQ        # 4
NKT = T // TK        # 16
NDT = D // 128       # 8

F32 = mybir.dt.float32
BF16 = mybir.dt.bfloat16
F8 = mybir.dt.float8e4
DR = mybir.MatmulPerfMode.DoubleRow
EXP = mybir.ActivationFunctionType.Exp
MULT = mybir.AluOpType.mult
BFNP = ml_dtypes.bfloat16
F8NP = ml_dtypes.float8_e4m3

LAST_RESULTS = None  # BassKernelResults of the most recent run (for test.py)


def _build_mha(tc, out_ap, in_aps):
    nc = tc.nc
    xt_d = in_aps["xt"]        # [128, 8, 2048] bf16
    wqk_d = in_aps["wqk"]      # [128, 8, 512] bf16
    bq_d = in_aps["bq"]        # [128, 2] f32
    wv_d = in_aps["wv"]        # [128, 8, 256] bf16
    wout_d = in_aps["wout"]    # [128, 2, 1024] bf16
    mask_d = in_aps["mask"]    # [128, 2, 896] bf16

    with ExitStack() as ctx:
        ctx.enter_context(nc.allow_low_precision(reason="bf16/fp8 pipeline"))
        const = ctx.enter_context(tc.tile_pool(name="const", bufs=1))
        big = ctx.enter_context(tc.tile_pool(name="big", bufs=1))

        xt = big.tile([128, NDT, T], BF16)
        # [32*(h%2)+j, h//2, A/B half, q/k, t]: per-head slices start at
        # partition 0 or 32 (base partition 96 is not addressable by the PE)
        qkt = big.tile([64, 2, 2, 2, T], F8)
        vp = big.tile([128, HL, NKT, 65], BF16)
        ot = big.tile([128, 2, T], BF16)

        wqk = const.tile([128, NDT, 512], BF16)
        wv = const.tile([128, NDT, 256], BF16)
        wout = const.tile([128, 2, 1024], BF16)
        bq = const.tile([128, 2], F32)
        mask = const.tile([128, 2, 896], BF16)

        # x^T dt-slices of the first token block and the matching weight
        # slices go out first (interleaved) so the PE can start the first
        # QKV matmul ~2.5us in; bulk transfers queue behind them.
        for dt in range(NDT):
            nc.sync.dma_start(wqk[:, dt, :], wqk_d[:, dt, :])
            nc.sync.dma_start(xt[:, dt, 0:TQ], xt_d[:, dt, 0:TQ])
        nc.sync.dma_start(bq[:], bq_d)
        nc.sync.dma_start(mask[:], mask_d)
        nc.sync.dma_start(wv[:], wv_d)
        for tt in range(1, NQT):
            nc.sync.dma_start(xt[:, :, tt * TQ:(tt + 1) * TQ],
                              xt_d[:, :, tt * TQ:(tt + 1) * TQ])
        nc.sync.dma_start(wout[:], wout_d)
        nc.gpsimd.memset(vp[:, :, :, 64], 1.0)

        with ExitStack() as pctx:
            ps = pctx.enter_context(tc.tile_pool(name="ps", bufs=2,
                                                 space="PSUM"))
            ps_av = pctx.enter_context(tc.tile_pool(name="ps_av", bufs=1,
                                                    space="PSUM"))
            ptp_p = pctx.enter_context(tc.tile_pool(name="ptp", bufs=2))
            nrm_p = pctx.enter_context(tc.tile_pool(name="nrm", bufs=2))
            ob_p = pctx.enter_context(tc.tile_pool(name="ob", bufs=4))

            def emit_fb(tt, fb):
                """One 128-col projection block (QA/QB/KA/KB) of token
                block tt, evacuated into the fp8 qkt layout."""
                pq = ps.tile([128, TQ], F32, tag="pq", name=f"pq_{tt}_{fb}")
                for dt in range(NDT):
                    nc.tensor.matmul(pq[:], wqk[:, dt, fb * 128:(fb + 1) * 128],
                                     xt[:, dt, tt * TQ:(tt + 1) * TQ],
                                     start=(dt == 0), stop=(dt == NDT - 1))
                half, qk = fb & 1, fb >> 1
                for hp in range(2):
                    dst = qkt[:, hp, half, qk, tt * TQ:(tt + 1) * TQ]
                    src = pq[64 * hp:64 * hp + 64, :]
                    if qk == 0:
                        nc.vector.tensor_scalar_add(
                            dst, src, bq[64 * hp:64 * hp + 64, half:half + 1])
                    else:
                        nc.vector.tensor_copy(dst, src)

            def emit_v(tt, ts):
                pv = ps.tile([128, 256], F32, tag="pq", name=f"pv_{tt}_{ts}")
                for dt in range(NDT):
                    nc.tensor.matmul(pv[:],
                                     xt[:, dt, (4 * tt + ts) * 128:
                                        (4 * tt + ts + 1) * 128],
                                     wv[:, dt, :],
                                     start=(dt == 0), stop=(dt == NDT - 1))
                nc.vector.tensor_copy(
                    vp[:, :, 4 * tt + ts, 0:64],
                    pv[:].rearrange("p (h e) -> p h e", e=HD))

            def emit_scores(a, qi, kt):
                """fp8 DoubleRow QK^T for head pair a at (qi, kt), one exp
                over both heads, diagonal mask multiply. Returns (ptp, c_lo).
                """
                rr = kt - 4 * qi
                c_lo = 0 if rr < 0 else 128 * rr
                s = ps.tile([128, 2, TQ], F32, tag="s", name=f"s_{a}_{qi}_{kt}")
                for i in range(2):
                    nc.tensor.matmul(
                        s[:, i, c_lo:],
                        qkt[32 * i:32 * i + 32, a, :, 1,
                            kt * TK:(kt + 1) * TK],
                        qkt[32 * i:32 * i + 32, a, :, 0,
                            qi * TQ + c_lo:(qi + 1) * TQ],
                        start=True, stop=True, perf_mode=DR)
                ptp = ptp_p.tile([128, 2, TQ], BF16, tag="pt",
                                 name=f"pt_{a}_{qi}_{kt}")
                nc.scalar.activation(ptp[:, :, c_lo:], s[:, :, c_lo:], EXP)
                if rr >= 0:
                    c0 = (3 - rr) * 128
                    nc.vector.tensor_tensor(
                        ptp[:, :, c_lo:c_lo + 128], ptp[:, :, c_lo:c_lo + 128],
                        mask[:, :, c0 + c_lo:c0 + c_lo + 128], MULT)
                return ptp, c_lo

            def emit_c(a, qi, fillers):
                av = [ps_av.tile([65, TQ], F32, tag=f"av{i}",
                                 name=f"av{i}_{a}_{qi}") for i in range(2)]
                nkt = 4 * qi + 4
                pts, c_lo = emit_scores(a, qi, 0)
                for kt in range(nkt):
                    nxt = (emit_scores(a, qi, kt + 1) if kt + 1 < nkt
                           else (None, 0))
                    n_pop = min(len(fillers),
                                max(1, -(-len(fillers) // (nkt - kt))))
                    for _ in range(n_pop):
                        fillers.popleft()()
                    for i in range(2):
                        h = 2 * a + i
                        nc.tensor.matmul(
                            av[i][0:65, c_lo:], vp[:, h, kt, :],
                            pts[:, i, c_lo:],
                            start=(kt == 0), stop=(kt == nkt - 1),
                            skip_group_check=True)
                    pts, c_lo = nxt
                # normalize: rows 0:64 are O^T, row 64 the softmax denominator
                for i in range(2):
                    rec = nrm_p.tile([1, TQ], F32, tag="rec",
                                     name=f"rec_{a}_{qi}_{i}")
                    nc.vector.reciprocal(rec[:], av[i][64:65, :])
                    bc = nrm_p.tile([64, TQ], F32, tag="bc",
                                    name=f"bc_{a}_{qi}_{i}")
                    nc.gpsimd.partition_broadcast(bc[:], rec[:])
                    nc.vector.tensor_tensor(
                        ot[64 * i:64 * i + 64, a, qi * TQ:(qi + 1) * TQ],
                        av[i][0:64, :], bc[:], MULT)

            def po_fillers(qi):
                def emit_po(ts, dt):
                    po = ps.tile([128, TQ], F32, tag="pq",
                                 name=f"po_{ts}_{dt}")
                    for ft in range(2):
                        nc.tensor.matmul(
                            po[:], ot[:, ft, ts * 128:(ts + 1) * 128],
                            wout[:, ft, dt * 512:(dt + 1) * 512],
                            start=(ft == 0), stop=(ft == 1))
                    ob = ob_p.tile([128, TQ], BF16, tag="ob",
                                   name=f"ob_{ts}_{dt}")
                    # GPSIMD cannot read PSUM; split evacuations DVE/Act
                    if (ts + dt) % 2 == 0:
                        nc.vector.tensor_copy(ob[:], po[:])
                    else:
                        nc.scalar.copy(ob[:], po[:])
                    nc.sync.dma_start(
                        out_ap[ts * 128:(ts + 1) * 128,
                               dt * 512:(dt + 1) * 512],
                        ob[:])
                return [
                    (lambda ts=ts, dt=dt: emit_po(ts, dt))
                    for ts in range(4 * qi, 4 * qi + 4) for dt in range(2)
                ]

            def qkv_fillers(tt):
                return ([(lambda fb=fb: emit_fb(tt, fb)) for fb in range(4)]
                        + [(lambda ts=ts: emit_v(tt, ts)) for ts in range(4)])

            # Block 0's QKV runs up front; thereafter block tt+1's QKV and
            # block tt-1's output projection slot into emit_c's filler gaps
            # so the PE never idles on the exp -> AV dependency.
            for f in qkv_fillers(0):
                f()
            for tt in range(NQT):
                fl = deque()
                qs = qkv_fillers(tt + 1) if tt + 1 < NQT else []
                pos = po_fillers(tt - 1) if tt >= 1 else []
                while qs or pos:
                    if qs:
                        fl.append(qs.pop(0))
                    if pos:
                        fl.append(pos.pop(0))
                emit_c(0, tt, fl)
                emit_c(1, tt, fl)
                while fl:
                    fl.popleft()()
            for f in po_fillers(NQT - 1):
                f()


_CACHE = {}


def _program():
    if "nc" in _CACHE:
        return _CACHE["nc"]
    nc = bacc.Bacc("TRN2", target_bir_lowering=False, debug=False)
    ins = {
        "xt": nc.dram_tensor("xt", [128, NDT, T], BF16,
                             kind="ExternalInput").ap(),
        "wqk": nc.dram_tensor("wqk", [128, NDT, 512], BF16,
                              kind="ExternalInput").ap(),
        "bq": nc.dram_tensor("bq", [128, 2], F32, kind="ExternalInput").ap(),
        "wv": nc.dram_tensor("wv", [128, NDT, 256], BF16,
                             kind="ExternalInput").ap(),
        "wout": nc.dram_tensor("wout", [128, 2, 1024], BF16,
                               kind="ExternalInput").ap(),
        "mask": nc.dram_tensor("mask", [128, 2, 896], BF16,
                               kind="ExternalInput").ap(),
    }
    out = nc.dram_tensor("out", [T, D], BF16, kind="ExternalOutput").ap()
    with tile.TileContext(nc) as tc:
        _build_mha(tc, out, ins)
    nc.compile()
    _CACHE["nc"] = nc
    return nc


def _in_maps(x, Wqkv, bqkv, Wout):
    x = np.asarray(x, dtype=np.float32)
    Wqkv = np.asarray(Wqkv, dtype=np.float32)
    bqkv = np.asarray(bqkv, dtype=np.float32)
    Wout = np.asarray(Wout, dtype=np.float32)
    scale = np.float32(1.0 / np.sqrt(HD))
    maskbase = (np.arange(128)[:, None] <= np.arange(896)[None, :] - 384)
    mask = np.ascontiguousarray(
        np.broadcast_to(maskbase[:, None, :], (128, 2, 896))).astype(BFNP)
    maps = []
    for c in range(NCORES):
        b, hg = c // 4, c % 4
        hs = [4 * hg + i for i in range(HL)]
        # [1024, 256] per-projection slices for this head group
        q_cols = np.concatenate(
            [Wqkv[:, h * HD:(h + 1) * HD] for h in hs], axis=1) * scale
        k_cols = np.concatenate(
            [Wqkv[:, D + h * HD:D + (h + 1) * HD] for h in hs], axis=1)
        v_cols = np.concatenate(
            [Wqkv[:, 2 * D + h * HD:2 * D + (h + 1) * HD] for h in hs], axis=1)
        # A/B half split: [1024, 4 heads, 2 halves, 32] -> QA|QB / KA|KB
        qr = q_cols.reshape(D, HL, 2, 32)
        kr = k_cols.reshape(D, HL, 2, 32)
        wqk_full = np.concatenate(
            [qr[:, :, 0].reshape(D, 128), qr[:, :, 1].reshape(D, 128),
             kr[:, :, 0].reshape(D, 128), kr[:, :, 1].reshape(D, 128)],
            axis=1)                                    # [1024, 512]
        wqk = np.ascontiguousarray(
            wqk_full.reshape(NDT, 128, 512).transpose(1, 0, 2)).astype(BFNP)
        bq_cols = np.concatenate(
            [bqkv[h * HD:(h + 1) * HD] for h in hs]) * scale
        bqr = bq_cols.reshape(HL, 2, 32)
        bq = np.ascontiguousarray(
            np.stack([bqr[:, 0].reshape(128), bqr[:, 1].reshape(128)],
                     axis=1)).astype(np.float32)       # [128, 2]
        wv = np.ascontiguousarray(
            v_cols.reshape(NDT, 128, 256).transpose(1, 0, 2)).astype(BFNP)
        wo = np.concatenate([Wout[h * HD:(h + 1) * HD, :] for h in hs], axis=0)
        wo = np.ascontiguousarray(
            wo.reshape(2, 128, D).transpose(1, 0, 2)).astype(BFNP)
        xt = np.ascontiguousarray(
            x[b].T.reshape(NDT, 128, T).transpose(1, 0, 2)).astype(BFNP)
        maps.append({
            "xt": xt,
            "wqk": wqk,
            "bq": bq,
            "wv": wv,
            "wout": wo,
            "mask": mask,
        })
    return maps


def kernel(x, Wqkv, bqkv, Wout, bout):
    global LAST_RESULTS
    nc = _program()
    maps = _in_maps(x, Wqkv, bqkv, Wout)
    res = run_bass_kernel_spmd(nc, maps, list(range(NCORES)))
    LAST_RESULTS = res
    bqkv = np.asarray(bqkv, dtype=np.float32)
    bout = np.asarray(bout, dtype=np.float32)
    # V bias folded: softmax weights sum to 1, so out += bv @ Wout exactly.
    bout_folded = bout + np.asarray(Wout, np.float32).T @ bqkv[2 * D:]
    out = np.empty((B, T, D), dtype=np.float32)
    for b in range(B):
        acc = np.asarray(res.results[4 * b]["out"], np.float32)
        for hg in range(1, 4):
            acc = acc + np.asarray(res.results[4 * b + hg]["out"], np.float32)
        out[b] = acc + bout_folded[None, :]
    return out


# revision 24
# speedup vs baseline: 1.2035x; 1.0934x over previous
"""Masked multi-head attention on 8 Trainium2 NeuronCores.

Sharding: core c = (b, hg) with b = c // 4, hg = c % 4. Each core computes the
full attention block for batch b restricted to heads [4*hg, 4*hg+4), including
its slice of the QKV projection and of the output projection. The host sums the
4 tensor-parallel partial outputs per batch and adds the (V-bias-folded) output
bias.

Numerics: QKV / AV / out-projection run in bf16 (PE rate 1.0 cycles/row, same
as fp32r, but half the DMA/SBUF), the QK^T score matmuls run in fp8e4m3 with
DoubleRow perf mode (0.5 cycles/row). K-bias is dropped (softmax row-shift
invariance), V-bias is folded into the output bias on the host, Q-bias is
applied during PSUM evacuation.

Layouts (per core):
  xt   [128, 8, 2048]  bf16  x^T: xt[p, dt, t] = x[t, 128*dt + p]
  wqk  [128, 8, 512]   bf16  col blocks QA|QB|KA|KB; block col 32*i + j is
                             head (4*hg + i), dims j / 32+j (A/B half).
                             Q pre-scaled by 1/sqrt(Hd).
  qkt  [128, 2, 2, T]  fp8   [32h+j, half, q/k, t] per-head Q^T/K^T
  wv   [128, 8, 256]   bf16
  vp   [128, 4, 16, 65] bf16 V tiles per (head, k-tile); col 64 = ones row
                             (softmax denominator trick)
  ot   [128, 2, T]     bf16  normalized O^T; rows 64*(h%2) + d at dim1 h//2
  wout [128, 2, 1024]  bf16
Shapes hardcoded for B=2, T=2048, D=1024, H=16, Hd=64.
"""

import numpy as np
import ml_dtypes
from collections import deque
from contextlib import ExitStack

import concourse.bass as bass
import concourse.bacc as bacc
import concourse.mybir as mybir
import concourse.tile as tile
from concourse.bass_utils import run_bass_kernel_spmd

B, T, D = 2, 2048, 1024
H, HD = 16, 64
HL = 4               # heads per core
NCORES = 8
TQ = 512             # query tile
TK = 128             # key tile
NQT = T // TQ        # 4
NKT = T // TK        # 16
NDT = D // 128       # 8

F32 = mybir.dt.float32
BF16 = mybir.dt.bfloat16
F8 = mybir.dt.float8e4
DR = mybir.MatmulPerfMode.DoubleRow
EXP = mybir.ActivationFunctionType.Exp
IDENT = mybir.ActivationFunctionType.Identity
MULT = mybir.AluOpType.mult
INV32 = 1.0 / 32.0
BFNP = ml_dtypes.bfloat16
F8NP = ml_dtypes.float8_e4m3

LAST_RESULTS = None  # BassKernelResults of the most recent run (for test.py)


def _build_mha(tc, out_ap, in_aps):
    nc = tc.nc
    xth_d = in_aps["xth"]      # [128, 8, 2048] fp8 hi
    xtl_d = in_aps["xtl"]      # [128, 8, 2048] fp8 lo (residual)
    wqkh_d = in_aps["wqkh"]    # [128, 4, 2, 512] fp8 hi
    wqkl_d = in_aps["wqkl"]    # [128, 4, 2, 512] fp8 lo
    bq_d = in_aps["bq"]        # [128, 2] f32
    wvh_d = in_aps["wvh"]      # [128, 4, 2, 256] fp8 hi
    wvl_d = in_aps["wvl"]      # [128, 4, 2, 256] fp8 lo
    wout_d = in_aps["wout"]    # [128, 2, 1024] bf16
    mask_d = in_aps["mask"]    # [128, 2, 896] bf16

    with ExitStack() as ctx:
        ctx.enter_context(nc.allow_low_precision(reason="bf16/fp8 pipeline"))
        const = ctx.enter_context(tc.tile_pool(name="const", bufs=1))
        big = ctx.enter_context(tc.tile_pool(name="big", bufs=1))

        xth = big.tile([128, NDT, T], F8)
        xtl = big.tile([128, NDT, T], F8)
        # [32*(h%2)+j, h//2, A/B half, q/k, t]: per-head slices start at
        # partition 0 or 32 (base partition 96 is not addressable by the PE)
        qkt = big.tile([64, 2, 2, 2, T], F8)
        vp = big.tile([128, HL, NKT, 65], BF16)
        ot = big.tile([128, 2, T], BF16)

        wqkh = const.tile([128, 4, 2, 512], F8)
        wqkl = const.tile([128, 4, 2, 512], F8)
        wvh = const.tile([128, 4, 2, 256], F8)
        wvl = const.tile([128, 4, 2, 256], F8)
        wout = const.tile([128, 2, 1024], BF16)
        bq = const.tile([128, 2], F32)
        mask = const.tile([128, 2, 896], BF16)

        # x^T dt-slices of the first token block and the matching weight
        # slices go out first (interleaved) so the PE can start the first
        # QKV matmul ~2.5us in; bulk transfers queue behind them.
        for j in range(4):
            nc.sync.dma_start(wqkh[:, j], wqkh_d[:, j])
            nc.sync.dma_start(xth[:, 2 * j:2 * j + 2, 0:TQ],
                              xth_d[:, 2 * j:2 * j + 2, 0:TQ])
        for j in range(4):
            nc.sync.dma_start(wqkl[:, j], wqkl_d[:, j])
            nc.sync.dma_start(xtl[:, 2 * j:2 * j + 2, 0:TQ],
                              xtl_d[:, 2 * j:2 * j + 2, 0:TQ])
        nc.sync.dma_start(bq[:], bq_d)
        nc.sync.dma_start(mask[:], mask_d)
        nc.sync.dma_start(wvh[:], wvh_d)
        nc.sync.dma_start(wvl[:], wvl_d)
        for tt in range(1, NQT):
            nc.sync.dma_start(xth[:, :, tt * TQ:(tt + 1) * TQ],
                              xth_d[:, :, tt * TQ:(tt + 1) * TQ])
            nc.sync.dma_start(xtl[:, :, tt * TQ:(tt + 1) * TQ],
                              xtl_d[:, :, tt * TQ:(tt + 1) * TQ])
        nc.sync.dma_start(wout[:], wout_d)
        nc.gpsimd.memset(vp[:, :, :, 64], 1.0)

        with ExitStack() as pctx:
            ps = pctx.enter_context(tc.tile_pool(name="ps", bufs=2,
                                                 space="PSUM"))
            ps_av = pctx.enter_context(tc.tile_pool(name="ps_av", bufs=1,
                                                    space="PSUM"))
            ptp_p = pctx.enter_context(tc.tile_pool(name="ptp", bufs=2))
            nrm_p = pctx.enter_context(tc.tile_pool(name="nrm", bufs=2))
            ob_p = pctx.enter_context(tc.tile_pool(name="ob", bufs=4))

            def emit_fb(tt, fb):
                """One 128-col projection block (QA/QB/KA/KB) of token block
                tt via split-fp8 DoubleRow (hi*Whi + lo*Whi + hi*Wlo),
                evacuated into the fp8 qkt layout."""
                pq = ps.tile([128, TQ], F32, tag="pq", name=f"pq_{tt}_{fb}")
                cols = slice(fb * 128, (fb + 1) * 128)
                tb = slice(tt * TQ, (tt + 1) * TQ)
                terms = [(wqkh, xth), (wqkh, xtl), (wqkl, xth)]
                for ti, (w, xx) in enumerate(terms):
                    for j in range(4):
                        nc.tensor.matmul(
                            pq[:], w[:, j, :, cols], xx[:, 2 * j:2 * j + 2, tb],
                            start=(ti == 0 and j == 0),
                            stop=(ti == 2 and j == 3), perf_mode=DR)
                half, qk = fb & 1, fb >> 1
                for hp in range(2):
                    dst = qkt[:, hp, half, qk, tt * TQ:(tt + 1) * TQ]
                    src = pq[64 * hp:64 * hp + 64, :]
                    if qk == 0:
                        # out = in/32 + bq on the Act engine (exp's table
                        # also holds Identity: no table reload)
                        nc.scalar.activation(
                            dst, src, IDENT,
                            bias=bq[64 * hp:64 * hp + 64, half:half + 1],
                            scale=INV32)
                    else:
                        nc.vector.tensor_scalar_mul(dst, src, INV32)

            def emit_v(tt, ts):
                pv = ps.tile([128, 256], F32, tag="pq", name=f"pv_{tt}_{ts}")
                tb = slice((4 * tt + ts) * 128, (4 * tt + ts + 1) * 128)
                terms = [(xth, wvh), (xtl, wvh), (xth, wvl)]
                for ti, (xx, w) in enumerate(terms):
                    for j in range(4):
                        nc.tensor.matmul(
                            pv[:], xx[:, 2 * j:2 * j + 2, tb], w[:, j],
                            start=(ti == 0 and j == 0),
                            stop=(ti == 2 and j == 3), perf_mode=DR)
                nc.vector.tensor_scalar_mul(
                    vp[:, :, 4 * tt + ts, 0:64],
                    pv[:].rearrange("p (h e) -> p h e", e=HD), INV32)

            def emit_scores(a, qi, kt):
                """fp8 DoubleRow QK^T for head pair a at (qi, kt), one exp
                over both heads, diagonal mask multiply. Returns (ptp, c_lo).
                """
                rr = kt - 4 * qi
                c_lo = 0 if rr < 0 else 128 * rr
                s = ps.tile([128, 2, TQ], F32, tag="s", name=f"s_{a}_{qi}_{kt}")
                for i in range(2):
                    nc.tensor.matmul(
                        s[:, i, c_lo:],
                        qkt[32 * i:32 * i + 32, a, :, 1,
                            kt * TK:(kt + 1) * TK],
                        qkt[32 * i:32 * i + 32, a, :, 0,
                            qi * TQ + c_lo:(qi + 1) * TQ],
                        start=True, stop=True, perf_mode=DR)
                ptp = ptp_p.tile([128, 2, TQ], BF16, tag="pt",
                                 name=f"pt_{a}_{qi}_{kt}")
                nc.scalar.activation(ptp[:, :, c_lo:], s[:, :, c_lo:], EXP)
                if rr >= 0:
                    c0 = (3 - rr) * 128
                    nc.vector.tensor_tensor(
                        ptp[:, :, c_lo:c_lo + 128], ptp[:, :, c_lo:c_lo + 128],
                        mask[:, :, c0 + c_lo:c0 + c_lo + 128], MULT)
                return ptp, c_lo

            def emit_c(a, qi, fillers, rem_iters):
                """rem_iters: kt iterations left in this tt including this
                pair's — paces the filler drain across both pairs."""
                av = [ps_av.tile([65, TQ], F32, tag=f"av{i}",
                                 name=f"av{i}_{a}_{qi}") for i in range(2)]
                nkt = 4 * qi + 4
                pts, c_lo = emit_scores(a, qi, 0)
                for kt in range(nkt):
                    nxt = (emit_scores(a, qi, kt + 1) if kt + 1 < nkt
                           else (None, 0))
                    n_pop = min(len(fillers),
                                -(-len(fillers) // max(1, rem_iters)))
                    rem_iters -= 1
                    for _ in range(n_pop):
                        fillers.popleft()()
                    for i in range(2):
                        h = 2 * a + i
                        nc.tensor.matmul(
                            av[i][0:65, c_lo:], vp[:, h, kt, :],
                            pts[:, i, c_lo:],
                            start=(kt == 0), stop=(kt == nkt - 1),
                            skip_group_check=True)
                    pts, c_lo = nxt
                # Evacuate av to SBUF immediately so the PSUM banks free for
                # the next pair; normalize off the critical path from SBUF.
                # Rows 0:64 are O^T, row 64 the softmax denominator. The last
                # block's banks are never reused — normalize from PSUM
                # directly to shorten the tail chain.
                last = qi == NQT - 1
                if last:
                    avs = av
                else:
                    avs = []
                    for i in range(2):
                        t = nrm_p.tile([65, TQ], F32, tag=f"avs{i}",
                                       name=f"avs{i}_{a}_{qi}")
                        nc.vector.tensor_copy(t[:], av[i][0:65, :])
                        avs.append(t)
                bcs = []
                for i in range(2):
                    rec = nrm_p.tile([1, TQ], F32, tag="rec",
                                     name=f"rec_{a}_{qi}_{i}")
                    nc.vector.reciprocal(rec[:], avs[i][64:65, :])
                    bc = nrm_p.tile([64, TQ], F32, tag="bc",
                                    name=f"bc_{a}_{qi}_{i}")
                    nc.gpsimd.partition_broadcast(bc[:], rec[:])
                    bcs.append(bc)
                if last:
                    # chunk per 128-token slice on DVE so the final
                    # out-projection units can start per-slice
                    for cs in range(4):
                        for i in range(2):
                            sl = slice(cs * 128, (cs + 1) * 128)
                            nc.vector.tensor_tensor(
                                ot[64 * i:64 * i + 64, a,
                                   qi * TQ + cs * 128:qi * TQ + (cs + 1) * 128],
                                avs[i][0:64, sl], bcs[i][:, sl], MULT)
                else:
                    for i in range(2):
                        nc.vector.tensor_tensor(
                            ot[64 * i:64 * i + 64, a,
                               qi * TQ:(qi + 1) * TQ],
                            avs[i][0:64, :], bcs[i][:], MULT)

            def po_fillers(qi):
                def emit_po(ts, dt):
                    po = ps.tile([128, TQ], F32, tag="pq",
                                 name=f"po_{ts}_{dt}")
                    for ft in range(2):
                        nc.tensor.matmul(
                            po[:], ot[:, ft, ts * 128:(ts + 1) * 128],
                            wout[:, ft, dt * 512:(dt + 1) * 512],
                            start=(ft == 0), stop=(ft == 1))
                    ob = ob_p.tile([128, TQ], BF16, tag="ob",
                                   name=f"ob_{ts}_{dt}")
                    nc.vector.tensor_copy(ob[:], po[:])
                    nc.sync.dma_start(
                        out_ap[ts * 128:(ts + 1) * 128,
                               dt * 512:(dt + 1) * 512],
                        ob[:])
                return [
                    (lambda ts=ts, dt=dt: emit_po(ts, dt))
                    for ts in range(4 * qi, 4 * qi + 4) for dt in range(2)
                ]

            def qkv_fillers(tt):
                return ([(lambda fb=fb: emit_fb(tt, fb)) for fb in range(4)]
                        + [(lambda ts=ts: emit_v(tt, ts)) for ts in range(4)])

            # Block 0's QKV runs up front; thereafter block tt+1's QKV and
            # block tt-1's output projection slot into emit_c's filler gaps
            # so the PE never idles on the exp -> AV dependency.
            for f in qkv_fillers(0):
                f()
            for tt in range(NQT):
                fl = deque()
                qs = qkv_fillers(tt + 1) if tt + 1 < NQT else []
                pos = po_fillers(tt - 1) if tt >= 1 else []
                while qs or pos:
                    if qs:
                        fl.append(qs.pop(0))
                    if pos:
                        fl.append(pos.pop(0))
                emit_c(0, tt, fl, 2 * (4 * tt + 4))
                emit_c(1, tt, fl, 4 * tt + 4)
                while fl:
                    fl.popleft()()
            for f in po_fillers(NQT - 1):
                f()


_CACHE = {}


def _program():
    if "nc" in _CACHE:
        return _CACHE["nc"]
    nc = bacc.Bacc("TRN2", target_bir_lowering=False, debug=False)
    ins = {
        "xth": nc.dram_tensor("xth", [128, NDT, T], F8,
                              kind="ExternalInput").ap(),
        "xtl": nc.dram_tensor("xtl", [128, NDT, T], F8,
                              kind="ExternalInput").ap(),
        "wqkh": nc.dram_tensor("wqkh", [128, 4, 2, 512], F8,
                               kind="ExternalInput").ap(),
        "wqkl": nc.dram_tensor("wqkl", [128, 4, 2, 512], F8,
                               kind="ExternalInput").ap(),
        "bq": nc.dram_tensor("bq", [128, 2], F32, kind="ExternalInput").ap(),
        "wvh": nc.dram_tensor("wvh", [128, 4, 2, 256], F8,
                              kind="ExternalInput").ap(),
        "wvl": nc.dram_tensor("wvl", [128, 4, 2, 256], F8,
                              kind="ExternalInput").ap(),
        "wout": nc.dram_tensor("wout", [128, 2, 1024], BF16,
                               kind="ExternalInput").ap(),
        "mask": nc.dram_tensor("mask", [128, 2, 896], BF16,
                               kind="ExternalInput").ap(),
    }
    out = nc.dram_tensor("out", [T, D], BF16, kind="ExternalOutput").ap()
    with tile.TileContext(nc) as tc:
        _build_mha(tc, out, ins)
    nc.compile()
    _CACHE["nc"] = nc
    return nc


def _in_maps(x, Wqkv, bqkv, Wout):
    x = np.asarray(x, dtype=np.float32)
    Wqkv = np.asarray(Wqkv, dtype=np.float32)
    bqkv = np.asarray(bqkv, dtype=np.float32)
    Wout = np.asarray(Wout, dtype=np.float32)
    scale = np.float32(1.0 / np.sqrt(HD))
    maskbase = (np.arange(128)[:, None] <= np.arange(896)[None, :] - 384)
    mask = np.ascontiguousarray(
        np.broadcast_to(maskbase[:, None, :], (128, 2, 896))).astype(BFNP)
    maps = []
    for c in range(NCORES):
        b, hg = c // 4, c % 4
        hs = [4 * hg + i for i in range(HL)]
        # [1024, 256] per-projection slices for this head group
        q_cols = np.concatenate(
            [Wqkv[:, h * HD:(h + 1) * HD] for h in hs], axis=1) * scale
        k_cols = np.concatenate(
            [Wqkv[:, D + h * HD:D + (h + 1) * HD] for h in hs], axis=1)
        v_cols = np.concatenate(
            [Wqkv[:, 2 * D + h * HD:2 * D + (h + 1) * HD] for h in hs], axis=1)
        # A/B half split: [1024, 4 heads, 2 halves, 32] -> QA|QB / KA|KB
        qr = q_cols.reshape(D, HL, 2, 32)
        kr = k_cols.reshape(D, HL, 2, 32)
        wqk_full = np.concatenate(
            [qr[:, :, 0].reshape(D, 128), qr[:, :, 1].reshape(D, 128),
             kr[:, :, 0].reshape(D, 128), kr[:, :, 1].reshape(D, 128)],
            axis=1)                                    # [1024, 512]

        def w_split(w):
            """x32 scale (keeps the lo residual in fp8 normal range), then
            fp8 hi/lo split in the [128, 4, 2, C] DoubleRow layout."""
            w = w * np.float32(32.0)
            C = w.shape[1]
            hi = w.astype(F8NP)
            lo = (w - hi.astype(np.float32)).astype(F8NP)
            def lay(a):
                return np.ascontiguousarray(
                    a.reshape(4, 2, 128, C).transpose(2, 0, 1, 3))
            return lay(hi), lay(lo)

        wqkh, wqkl = w_split(wqk_full)
        wvh, wvl = w_split(v_cols)
        bq_cols = np.concatenate(
            [bqkv[h * HD:(h + 1) * HD] for h in hs]) * scale
        bqr = bq_cols.reshape(HL, 2, 32)
        bq = np.ascontiguousarray(
            np.stack([bqr[:, 0].reshape(128), bqr[:, 1].reshape(128)],
                     axis=1)).astype(np.float32)       # [128, 2]
        wo = np.concatenate([Wout[h * HD:(h + 1) * HD, :] for h in hs], axis=0)
        wo = np.ascontiguousarray(
            wo.reshape(2, 128, D).transpose(1, 0, 2)).astype(BFNP)
        xtf = np.ascontiguousarray(
            x[b].T.reshape(NDT, 128, T).transpose(1, 0, 2))
        xth = xtf.astype(F8NP)
        xtl = (xtf - xth.astype(np.float32)).astype(F8NP)
        maps.append({
            "xth": xth,
            "xtl": xtl,
            "wqkh": wqkh,
            "wqkl": wqkl,
            "bq": bq,
            "wvh": wvh,
            "wvl": wvl,
            "wout": wo,
            "mask": mask,
        })
    return maps


def kernel(x, Wqkv, bqkv, Wout, bout):
    global LAST_RESULTS
    nc = _program()
    maps = _in_maps(x, Wqkv, bqkv, Wout)
    res = run_bass_kernel_spmd(nc, maps, list(range(NCORES)))
    LAST_RESULTS = res
    bqkv = np.asarray(bqkv, dtype=np.float32)
    bout = np.asarray(bout, dtype=np.float32)
    # V bias folded: softmax weights sum to 1, so out += bv @ Wout exactly.
    bout_folded = bout + np.asarray(Wout, np.float32).T @ bqkv[2 * D:]
    out = np.empty((B, T, D), dtype=np.float32)
    for b in range(B):
        acc = np.asarray(res.results[4 * b]["out"], np.float32)
        for hg in range(1, 4):
            acc = acc + np.asarray(res.results[4 * b + hg]["out"], np.float32)
        out[b] = acc + bout_folded[None, :]
    return out


# revision 31
# speedup vs baseline: 1.3315x; 1.1063x over previous
"""Masked multi-head attention on 8 Trainium2 NeuronCores.

Sharding: core c = (b, hg) with b = c // 4, hg = c % 4. Each core computes the
full attention block for batch b restricted to heads [4*hg, 4*hg+4), including
its slice of the QKV projection and of the output projection. The host sums the
4 tensor-parallel partial outputs per batch and adds the (V-bias-folded) output
bias.

Numerics: QKV / AV / out-projection run in bf16 (PE rate 1.0 cycles/row, same
as fp32r, but half the DMA/SBUF), the QK^T score matmuls run in fp8e4m3 with
DoubleRow perf mode (0.5 cycles/row). K-bias is dropped (softmax row-shift
invariance), V-bias is folded into the output bias on the host, Q-bias is
applied during PSUM evacuation.

Layouts (per core):
  xt   [128, 8, 2048]  bf16  x^T: xt[p, dt, t] = x[t, 128*dt + p]
  wqk  [128, 8, 512]   bf16  col blocks QA|QB|KA|KB; block col 32*i + j is
                             head (4*hg + i), dims j / 32+j (A/B half).
                             Q pre-scaled by 1/sqrt(Hd).
  qkt  [128, 2, 2, T]  fp8   [32h+j, half, q/k, t] per-head Q^T/K^T
  wv   [128, 8, 256]   bf16
  vp   [128, 4, 16, 65] bf16 V tiles per (head, k-tile); col 64 = ones row
                             (softmax denominator trick)
  ot   [128, 2, T]     bf16  normalized O^T; rows 64*(h%2) + d at dim1 h//2
  wout [128, 2, 1024]  bf16
Shapes hardcoded for B=2, T=2048, D=1024, H=16, Hd=64.
"""

import numpy as np
import ml_dtypes
from collections import deque
from contextlib import ExitStack

import concourse.bass as bass
import concourse.bacc as bacc
import concourse.mybir as mybir
import concourse.tile as tile
from concourse.bass_utils import run_bass_kernel_spmd

B, T, D = 2, 2048, 1024
H, HD = 16, 64
HL = 4               # heads per core
NCORES = 8
TQ = 512             # query tile
TK = 128             # key tile
NQT = T // TQ        # 4
NKT = T // TK        # 16
NDT = D // 128       # 8

F32 = mybir.dt.float32
BF16 = mybir.dt.bfloat16
F8 = mybir.dt.float8e4
DR = mybir.MatmulPerfMode.DoubleRow
EXP = mybir.ActivationFunctionType.Exp
IDENT = mybir.ActivationFunctionType.Identity
MULT = mybir.AluOpType.mult
INV32 = 1.0 / 32.0
BFNP = ml_dtypes.bfloat16
F8NP = ml_dtypes.float8_e4m3

LAST_RESULTS = None  # BassKernelResults of the most recent run (for test.py)


def _build_mha(tc, out_ap, in_aps):
    nc = tc.nc
    xth_d = in_aps["xth"]      # [128, 8, 2048] fp8 hi
    xtl_d = in_aps["xtl"]      # [128, 8, 2048] fp8 lo (residual)
    wqkh_d = in_aps["wqkh"]    # [128, 4, 2, 512] fp8 hi
    wqkl_d = in_aps["wqkl"]    # [128, 4, 2, 512] fp8 lo
    bq_d = in_aps["bq"]        # [128, 2] f32
    wvh_d = in_aps["wvh"]      # [128, 4, 2, 256] fp8 hi
    wvl_d = in_aps["wvl"]      # [128, 4, 2, 256] fp8 lo
    wout_d = in_aps["wout"]    # [128, 2, 1024] bf16
    mask_d = in_aps["mask"]    # [128, 2, 896] bf16

    with ExitStack() as ctx:
        ctx.enter_context(nc.allow_low_precision(reason="bf16/fp8 pipeline"))
        const = ctx.enter_context(tc.tile_pool(name="const", bufs=1))
        big = ctx.enter_context(tc.tile_pool(name="big", bufs=1))

        xth = big.tile([128, NDT, T], F8)
        xtl = big.tile([128, NDT, T], F8)
        # [32*(h%2)+j, h//2, A/B half, q/k, t]: per-head slices start at
        # partition 0 or 32 (base partition 96 is not addressable by the PE)
        qkt = big.tile([64, 2, 2, 2, T], F8)
        vp = big.tile([128, HL, NKT, 65], BF16)
        ot = big.tile([128, 2, T], BF16)

        wqkh = const.tile([128, 4, 2, 512], F8)
        wqkl = const.tile([128, 4, 2, 512], F8)
        wvh = const.tile([128, 4, 2, 256], F8)
        wvl = const.tile([128, 4, 2, 256], F8)
        wout = const.tile([128, 2, 1024], BF16)
        bq = const.tile([128, 2], F32)
        mask = const.tile([128, 2, 896], BF16)

        # HWDGE serializes ~625ns per DMA instruction: use few, large
        # transfers. Block-0 QKV inputs first, bulk x behind, weights last.
        nc.sync.dma_start(wqkh[:], wqkh_d)
        nc.sync.dma_start(xth[:, :, 0:TQ], xth_d[:, :, 0:TQ])
        nc.sync.dma_start(wqkl[:], wqkl_d)
        nc.sync.dma_start(xtl[:, :, 0:TQ], xtl_d[:, :, 0:TQ])
        nc.sync.dma_start(bq[:], bq_d)
        nc.sync.dma_start(mask[:], mask_d)
        nc.sync.dma_start(wvh[:], wvh_d)
        nc.sync.dma_start(wvl[:], wvl_d)
        nc.sync.dma_start(xth[:, :, TQ:], xth_d[:, :, TQ:])
        nc.sync.dma_start(xtl[:, :, TQ:], xtl_d[:, :, TQ:])
        nc.sync.dma_start(wout[:], wout_d)
        nc.gpsimd.memset(vp[:, :, :, 64], 1.0)

        with ExitStack() as pctx:
            ps = pctx.enter_context(tc.tile_pool(name="ps", bufs=2,
                                                 space="PSUM"))
            ps_av = pctx.enter_context(tc.tile_pool(name="ps_av", bufs=1,
                                                    space="PSUM"))
            ptp_p = pctx.enter_context(tc.tile_pool(name="ptp", bufs=2))
            nrm_p = pctx.enter_context(tc.tile_pool(name="nrm", bufs=2))
            ob_p = pctx.enter_context(tc.tile_pool(name="ob", bufs=4))

            def emit_fb(tt, fb):
                """One 128-col projection block (QA/QB/KA/KB) of token block
                tt via split-fp8 DoubleRow (hi*Whi + lo*Whi + hi*Wlo),
                evacuated into the fp8 qkt layout."""
                pq = ps.tile([128, TQ], F32, tag="pq", name=f"pq_{tt}_{fb}")
                cols = slice(fb * 128, (fb + 1) * 128)
                tb = slice(tt * TQ, (tt + 1) * TQ)
                terms = [(wqkh, xth), (wqkl, xth), (wqkh, xtl)]
                for ti, (w, xx) in enumerate(terms):
                    for j in range(4):
                        nc.tensor.matmul(
                            pq[:], w[:, j, :, cols], xx[:, 2 * j:2 * j + 2, tb],
                            start=(ti == 0 and j == 0),
                            stop=(ti == 2 and j == 3), perf_mode=DR)
                half, qk = fb & 1, fb >> 1
                # the two 64-row halves evacuate on different engines in
                # parallel so the PSUM bank frees in one evac latency
                for hp in range(2):
                    dst = qkt[:, hp, half, qk, tt * TQ:(tt + 1) * TQ]
                    src = pq[64 * hp:64 * hp + 64, :]
                    bqs = bq[64 * hp:64 * hp + 64, half:half + 1]
                    if qk == 0:
                        if hp == 0:
                            # out = in/32 + bq (Identity shares exp's table)
                            nc.scalar.activation(dst, src, IDENT, bias=bqs,
                                                 scale=INV32)
                        else:
                            nc.vector.tensor_scalar(
                                dst, src, INV32, bqs,
                                mybir.AluOpType.mult, mybir.AluOpType.add)
                    else:
                        if hp == 0:
                            nc.scalar.mul(dst, src, INV32)
                        else:
                            nc.vector.tensor_scalar_mul(dst, src, INV32)

            def emit_v(tt, ts):
                pv = ps.tile([128, 256], F32, tag="pq", name=f"pv_{tt}_{ts}")
                tb = slice((4 * tt + ts) * 128, (4 * tt + ts + 1) * 128)
                terms = [(xth, wvh), (xth, wvl), (xtl, wvh)]
                for ti, (xx, w) in enumerate(terms):
                    for j in range(4):
                        nc.tensor.matmul(
                            pv[:], xx[:, 2 * j:2 * j + 2, tb], w[:, j],
                            start=(ti == 0 and j == 0),
                            stop=(ti == 2 and j == 3), perf_mode=DR)
                nc.vector.tensor_scalar_mul(
                    vp[:, :, 4 * tt + ts, 0:64],
                    pv[:].rearrange("p (h e) -> p h e", e=HD), INV32)

            def emit_scores(a, qi, kt):
                """fp8 DoubleRow QK^T for head pair a at (qi, kt), one exp
                over both heads, diagonal mask multiply. Returns (ptp, c_lo).
                """
                rr = kt - 4 * qi
                c_lo = 0 if rr < 0 else 128 * rr
                s = ps.tile([128, 2, TQ], F32, tag="s", name=f"s_{a}_{qi}_{kt}")
                for i in range(2):
                    nc.tensor.matmul(
                        s[:, i, c_lo:],
                        qkt[32 * i:32 * i + 32, a, :, 1,
                            kt * TK:(kt + 1) * TK],
                        qkt[32 * i:32 * i + 32, a, :, 0,
                            qi * TQ + c_lo:(qi + 1) * TQ],
                        start=True, stop=True, perf_mode=DR)
                ptp = ptp_p.tile([128, 2, TQ], BF16, tag="pt",
                                 name=f"pt_{a}_{qi}_{kt}")
                nc.scalar.activation(ptp[:, :, c_lo:], s[:, :, c_lo:], EXP)
                if rr >= 0:
                    c0 = (3 - rr) * 128
                    nc.vector.tensor_tensor(
                        ptp[:, :, c_lo:c_lo + 128], ptp[:, :, c_lo:c_lo + 128],
                        mask[:, :, c0 + c_lo:c0 + c_lo + 128], MULT)
                return ptp, c_lo

            def emit_c(a, qi, fillers, rem_iters):
                """rem_iters: kt iterations left in this tt including this
                pair's — paces the filler drain across both pairs."""
                av = [ps_av.tile([65, TQ], F32, tag=f"av{i}",
                                 name=f"av{i}_{a}_{qi}") for i in range(2)]
                nkt = 4 * qi + 4
                pts, c_lo = emit_scores(a, qi, 0)
                for kt in range(nkt):
                    nxt = (emit_scores(a, qi, kt + 1) if kt + 1 < nkt
                           else (None, 0))
                    n_pop = min(len(fillers),
                                -(-len(fillers) // max(1, rem_iters)))
                    rem_iters -= 1
                    for _ in range(n_pop):
                        fillers.popleft()()
                    for i in range(2):
                        h = 2 * a + i
                        nc.tensor.matmul(
                            av[i][0:65, c_lo:], vp[:, h, kt, :],
                            pts[:, i, c_lo:],
                            start=(kt == 0), stop=(kt == nkt - 1),
                            skip_group_check=True)
                    pts, c_lo = nxt
                # Evacuate av to SBUF immediately so the PSUM banks free for
                # the next pair; normalize off the critical path from SBUF.
                # Rows 0:64 are O^T, row 64 the softmax denominator. The last
                # block's banks are never reused — normalize from PSUM
                # directly to shorten the tail chain.
                last = qi == NQT - 1
                if last:
                    avs = av
                else:
                    avs = []
                    for i in range(2):
                        t = nrm_p.tile([65, TQ], F32, tag=f"avs{i}",
                                       name=f"avs{i}_{a}_{qi}")
                        nc.vector.tensor_copy(t[:], av[i][0:65, :])
                        avs.append(t)
                bcs = []
                for i in range(2):
                    rec = nrm_p.tile([1, TQ], F32, tag="rec",
                                     name=f"rec_{a}_{qi}_{i}")
                    nc.vector.reciprocal(rec[:], avs[i][64:65, :])
                    bc = nrm_p.tile([64, TQ], F32, tag="bc",
                                    name=f"bc_{a}_{qi}_{i}")
                    nc.gpsimd.partition_broadcast(bc[:], rec[:])
                    bcs.append(bc)
                if last:
                    # chunk per 128-token slice on DVE so the final
                    # out-projection units can start per-slice
                    for cs in range(4):
                        for i in range(2):
                            sl = slice(cs * 128, (cs + 1) * 128)
                            nc.vector.tensor_tensor(
                                ot[64 * i:64 * i + 64, a,
                                   qi * TQ + cs * 128:qi * TQ + (cs + 1) * 128],
                                avs[i][0:64, sl], bcs[i][:, sl], MULT)
                else:
                    for i in range(2):
                        nc.vector.tensor_tensor(
                            ot[64 * i:64 * i + 64, a,
                               qi * TQ:(qi + 1) * TQ],
                            avs[i][0:64, :], bcs[i][:], MULT)

            def po_fillers(qi):
                def emit_po(ts, dt, ob):
                    po = ps.tile([128, TQ], F32, tag="pq",
                                 name=f"po_{ts}_{dt}")
                    for ft in range(2):
                        nc.tensor.matmul(
                            po[:], ot[:, ft, ts * 128:(ts + 1) * 128],
                            wout[:, ft, dt * 512:(dt + 1) * 512],
                            start=(ft == 0), stop=(ft == 1))
                    nc.vector.tensor_copy(ob[:, dt * 512:(dt + 1) * 512],
                                          po[:])
                    if dt == 1:
                        # one DMA per 128-token row (HWDGE overhead halves)
                        nc.sync.dma_start(out_ap[ts * 128:(ts + 1) * 128, :],
                                          ob[:])
                obs = {}
                def unit(ts, dt):
                    if dt == 0:
                        obs[ts] = ob_p.tile([128, 2 * TQ], BF16, tag="ob",
                                            name=f"ob_{ts}")
                    emit_po(ts, dt, obs[ts])
                return [
                    (lambda ts=ts, dt=dt: unit(ts, dt))
                    for ts in range(4 * qi, 4 * qi + 4) for dt in range(2)
                ]

            def qkv_fillers(tt):
                return ([(lambda fb=fb: emit_fb(tt, fb)) for fb in range(4)]
                        + [(lambda ts=ts: emit_v(tt, ts)) for ts in range(4)])

            # Block 0's Q/K projections run up front; everything else (V(0),
            # later blocks' QKV, all deferrable out-projections) slots into
            # emit_c's filler gaps so the PE never idles on the exp -> AV
            # dependency. The out-projections all land in the last block,
            # whose long kt loops otherwise run out of filler work.
            for fb in range(4):
                emit_fb(0, fb)
            for tt in range(NQT):
                fl = deque()
                if tt == 0:
                    fl.extend([(lambda ts=ts: emit_v(0, ts))
                               for ts in range(4)])
                if tt + 1 < NQT:
                    fl.extend(qkv_fillers(tt + 1))
                else:
                    for qi in range(NQT - 1):
                        fl.extend(po_fillers(qi))
                emit_c(0, tt, fl, 2 * (4 * tt + 4))
                emit_c(1, tt, fl, 4 * tt + 4)
                while fl:
                    fl.popleft()()
            for f in po_fillers(NQT - 1):
                f()


_CACHE = {}


def _program():
    if "nc" in _CACHE:
        return _CACHE["nc"]
    nc = bacc.Bacc("TRN2", target_bir_lowering=False, debug=False)
    ins = {
        "xth": nc.dram_tensor("xth", [128, NDT, T], F8,
                              kind="ExternalInput").ap(),
        "xtl": nc.dram_tensor("xtl", [128, NDT, T], F8,
                              kind="ExternalInput").ap(),
        "wqkh": nc.dram_tensor("wqkh", [128, 4, 2, 512], F8,
                               kind="ExternalInput").ap(),
        "wqkl": nc.dram_tensor("wqkl", [128, 4, 2, 512], F8,
                               kind="ExternalInput").ap(),
        "bq": nc.dram_tensor("bq", [128, 2], F32, kind="ExternalInput").ap(),
        "wvh": nc.dram_tensor("wvh", [128, 4, 2, 256], F8,
                              kind="ExternalInput").ap(),
        "wvl": nc.dram_tensor("wvl", [128, 4, 2, 256], F8,
                              kind="ExternalInput").ap(),
        "wout": nc.dram_tensor("wout", [128, 2, 1024], BF16,
                               kind="ExternalInput").ap(),
        "mask": nc.dram_tensor("mask", [128, 2, 896], BF16,
                               kind="ExternalInput").ap(),
    }
    out = nc.dram_tensor("out", [T, D], BF16, kind="ExternalOutput").ap()
    with tile.TileContext(nc) as tc:
        _build_mha(tc, out, ins)
    nc.compile()
    _CACHE["nc"] = nc
    return nc


def _in_maps(x, Wqkv, bqkv, Wout):
    x = np.asarray(x, dtype=np.float32)
    Wqkv = np.asarray(Wqkv, dtype=np.float32)
    bqkv = np.asarray(bqkv, dtype=np.float32)
    Wout = np.asarray(Wout, dtype=np.float32)
    scale = np.float32(1.0 / np.sqrt(HD))
    maskbase = (np.arange(128)[:, None] <= np.arange(896)[None, :] - 384)
    mask = np.ascontiguousarray(
        np.broadcast_to(maskbase[:, None, :], (128, 2, 896))).astype(BFNP)
    maps = []
    for c in range(NCORES):
        b, hg = c // 4, c % 4
        hs = [4 * hg + i for i in range(HL)]
        # [1024, 256] per-projection slices for this head group
        q_cols = np.concatenate(
            [Wqkv[:, h * HD:(h + 1) * HD] for h in hs], axis=1) * scale
        k_cols = np.concatenate(
            [Wqkv[:, D + h * HD:D + (h + 1) * HD] for h in hs], axis=1)
        v_cols = np.concatenate(
            [Wqkv[:, 2 * D + h * HD:2 * D + (h + 1) * HD] for h in hs], axis=1)
        # A/B half split: [1024, 4 heads, 2 halves, 32] -> QA|QB / KA|KB
        qr = q_cols.reshape(D, HL, 2, 32)
        kr = k_cols.reshape(D, HL, 2, 32)
        wqk_full = np.concatenate(
            [qr[:, :, 0].reshape(D, 128), qr[:, :, 1].reshape(D, 128),
             kr[:, :, 0].reshape(D, 128), kr[:, :, 1].reshape(D, 128)],
            axis=1)                                    # [1024, 512]

        def w_split(w):
            """x32 scale (keeps the lo residual in fp8 normal range), then
            fp8 hi/lo split in the [128, 4, 2, C] DoubleRow layout."""
            w = w * np.float32(32.0)
            C = w.shape[1]
            hi = w.astype(F8NP)
            lo = (w - hi.astype(np.float32)).astype(F8NP)
            def lay(a):
                return np.ascontiguousarray(
                    a.reshape(4, 2, 128, C).transpose(2, 0, 1, 3))
            return lay(hi), lay(lo)

        wqkh, wqkl = w_split(wqk_full)
        wvh, wvl = w_split(v_cols)
        bq_cols = np.concatenate(
            [bqkv[h * HD:(h + 1) * HD] for h in hs]) * scale
        bqr = bq_cols.reshape(HL, 2, 32)
        bq = np.ascontiguousarray(
            np.stack([bqr[:, 0].reshape(128), bqr[:, 1].reshape(128)],
                     axis=1)).astype(np.float32)       # [128, 2]
        wo = np.concatenate([Wout[h * HD:(h + 1) * HD, :] for h in hs], axis=0)
        wo = np.ascontiguousarray(
            wo.reshape(2, 128, D).transpose(1, 0, 2)).astype(BFNP)
        xtf = np.ascontiguousarray(
            x[b].T.reshape(NDT, 128, T).transpose(1, 0, 2))
        xth = xtf.astype(F8NP)
        xtl = (xtf - xth.astype(np.float32)).astype(F8NP)
        maps.append({
            "xth": xth,
            "xtl": xtl,
            "wqkh": wqkh,
            "wqkl": wqkl,
            "bq": bq,
            "wvh": wvh,
            "wvl": wvl,
            "wout": wo,
            "mask": mask,
        })
    return maps


def kernel(x, Wqkv, bqkv, Wout, bout):
    global LAST_RESULTS
    nc = _program()
    maps = _in_maps(x, Wqkv, bqkv, Wout)
    res = run_bass_kernel_spmd(nc, maps, list(range(NCORES)))
    LAST_RESULTS = res
    bqkv = np.asarray(bqkv, dtype=np.float32)
    bout = np.asarray(bout, dtype=np.float32)
    # V bias folded: softmax weights sum to 1, so out += bv @ Wout exactly.
    bout_folded = bout + np.asarray(Wout, np.float32).T @ bqkv[2 * D:]
    out = np.empty((B, T, D), dtype=np.float32)
    for b in range(B):
        acc = np.asarray(res.results[4 * b]["out"], np.float32)
        for hg in range(1, 4):
            acc = acc + np.asarray(res.results[4 * b + hg]["out"], np.float32)
        out[b] = acc + bout_folded[None, :]
    return out


# revision 48
# speedup vs baseline: 1.3898x; 1.0438x over previous
"""Masked multi-head attention on 8 Trainium2 NeuronCores.

Sharding: core c = (b, hg) with b = c // 4, hg = c % 4. Each core computes the
full attention block for batch b restricted to heads [4*hg, 4*hg+4), including
its slice of the QKV projection and of the output projection. The host sums the
4 tensor-parallel partial outputs per batch and adds the (V-bias-folded) output
bias.

Numerics: QKV / AV / out-projection run in bf16 (PE rate 1.0 cycles/row, same
as fp32r, but half the DMA/SBUF), the QK^T score matmuls run in fp8e4m3 with
DoubleRow perf mode (0.5 cycles/row). K-bias is dropped (softmax row-shift
invariance), V-bias is folded into the output bias on the host, Q-bias is
applied during PSUM evacuation.

Layouts (per core):
  xt   [128, 8, 2048]  bf16  x^T: xt[p, dt, t] = x[t, 128*dt + p]
  wqk  [128, 8, 512]   bf16  col blocks QA|QB|KA|KB; block col 32*i + j is
                             head (4*hg + i), dims j / 32+j (A/B half).
                             Q pre-scaled by 1/sqrt(Hd).
  qkt  [128, 2, 2, T]  fp8   [32h+j, half, q/k, t] per-head Q^T/K^T
  wv   [128, 8, 256]   bf16
  vp   [128, 4, 16, 65] bf16 V tiles per (head, k-tile); col 64 = ones row
                             (softmax denominator trick)
  ot   [128, 2, T]     bf16  normalized O^T; rows 64*(h%2) + d at dim1 h//2
  wout [128, 2, 1024]  bf16
Shapes hardcoded for B=2, T=2048, D=1024, H=16, Hd=64.
"""

import numpy as np
import ml_dtypes
from collections import deque
from contextlib import ExitStack

import concourse.bass as bass
import concourse.bacc as bacc
import concourse.mybir as mybir
import concourse.tile as tile
from concourse.bass_utils import run_bass_kernel_spmd

B, T, D = 2, 2048, 1024
H, HD = 16, 64
HL = 4               # heads per core
NCORES = 8
TQ = 512             # query tile
TK = 128             # key tile
NQT = T // TQ        # 4
NKT = T // TK        # 16
NDT = D // 128       # 8

F32 = mybir.dt.float32
BF16 = mybir.dt.bfloat16
F8 = mybir.dt.float8e4
DR = mybir.MatmulPerfMode.DoubleRow
EXP = mybir.ActivationFunctionType.Exp
IDENT = mybir.ActivationFunctionType.Identity
MULT = mybir.AluOpType.mult
INV32 = 1.0 / 32.0
BFNP = ml_dtypes.bfloat16
F8NP = ml_dtypes.float8_e4m3

LAST_RESULTS = None  # BassKernelResults of the most recent run (for test.py)


def _build_mha(tc, out_ap, in_aps):
    nc = tc.nc
    xth_d = in_aps["xth"]      # [128, 8, 2048] fp8 hi
    xtl_d = in_aps["xtl"]      # [128, 8, 2048] fp8 lo (residual)
    wqkh_d = in_aps["wqkh"]    # [128, 4, 2, 512] fp8 hi
    wqkl_d = in_aps["wqkl"]    # [128, 4, 2, 512] fp8 lo
    bq_d = in_aps["bq"]        # [128, 2] f32
    wvh_d = in_aps["wvh"]      # [128, 4, 2, 256] fp8 hi
    wvl_d = in_aps["wvl"]      # [128, 4, 2, 256] fp8 lo
    wout_d = in_aps["wout"]    # [128, 2, 1024] bf16
    mask_d = in_aps["mask"]    # [128, 2, 896] bf16

    with ExitStack() as ctx:
        ctx.enter_context(nc.allow_low_precision(reason="bf16/fp8 pipeline"))
        const = ctx.enter_context(tc.tile_pool(name="const", bufs=1))
        big = ctx.enter_context(tc.tile_pool(name="big", bufs=1))

        xth = big.tile([128, NDT, T], F8)
        xtl = big.tile([128, NDT, T], F8)
        # [32*(h%2)+j, h//2, A/B half, q/k, t]: per-head slices start at
        # partition 0 or 32 (base partition 96 is not addressable by the PE)
        qkt = big.tile([64, 2, 2, 2, T], F8)
        vp = big.tile([128, HL, NKT, 65], BF16)
        ot = big.tile([128, 2, T], BF16)

        wqkh = const.tile([128, 4, 2, 512], F8)
        wqkl = const.tile([128, 4, 2, 512], F8)
        wvh = const.tile([128, 4, 2, 256], F8)
        wvl = const.tile([128, 4, 2, 256], F8)
        wout = const.tile([128, 2, 1024], BF16)
        bq = const.tile([128, 2], F32)
        mask = const.tile([128, 2, 896], BF16)

        # HWDGE serializes ~625ns per DMA instruction: use few, large
        # transfers. Block-0 QKV inputs first, bulk x behind, weights last.
        nc.sync.dma_start(wqkh[:], wqkh_d)
        nc.sync.dma_start(xth[:, :, 0:TQ], xth_d[:, :, 0:TQ])
        nc.sync.dma_start(wqkl[:], wqkl_d)
        nc.sync.dma_start(xtl[:, :, 0:TQ], xtl_d[:, :, 0:TQ])
        nc.sync.dma_start(bq[:], bq_d)
        nc.sync.dma_start(mask[:], mask_d)
        nc.sync.dma_start(wvh[:], wvh_d)
        nc.sync.dma_start(wvl[:], wvl_d)
        nc.sync.dma_start(xth[:, :, TQ:], xth_d[:, :, TQ:])
        nc.sync.dma_start(xtl[:, :, TQ:], xtl_d[:, :, TQ:])
        nc.sync.dma_start(wout[:], wout_d)
        nc.gpsimd.memset(vp[:, :, :, 64], 1.0)

        with ExitStack() as pctx:
            ps = pctx.enter_context(tc.tile_pool(name="ps", bufs=2,
                                                 space="PSUM"))
            ps_av = pctx.enter_context(tc.tile_pool(name="ps_av", bufs=1,
                                                    space="PSUM"))
            ptp_p = pctx.enter_context(tc.tile_pool(name="ptp", bufs=8))
            nrm_p = pctx.enter_context(tc.tile_pool(name="nrm", bufs=4))
            ob_p = pctx.enter_context(tc.tile_pool(name="ob", bufs=6))

            def emit_fb(tt, fb):
                """One 128-col projection block (QA/QB/KA/KB) of token block
                tt via split-fp8 DoubleRow (hi*Whi + lo*Whi + hi*Wlo),
                evacuated into the fp8 qkt layout."""
                pq = ps.tile([128, TQ], F32, tag="pq", name=f"pq_{tt}_{fb}")
                cols = slice(fb * 128, (fb + 1) * 128)
                tb = slice(tt * TQ, (tt + 1) * TQ)
                terms = [(wqkh, xth), (wqkl, xth), (wqkh, xtl)]
                for ti, (w, xx) in enumerate(terms):
                    for j in range(4):
                        nc.tensor.matmul(
                            pq[:], w[:, j, :, cols], xx[:, 2 * j:2 * j + 2, tb],
                            start=(ti == 0 and j == 0),
                            stop=(ti == 2 and j == 3), perf_mode=DR)
                half, qk = fb & 1, fb >> 1
                # the two 64-row halves evacuate on different engines in
                # parallel so the PSUM bank frees in one evac latency
                for hp in range(2):
                    dst = qkt[:, hp, half, qk, tt * TQ:(tt + 1) * TQ]
                    src = pq[64 * hp:64 * hp + 64, :]
                    bqs = bq[64 * hp:64 * hp + 64, half:half + 1]
                    if qk == 0:
                        if hp == 0:
                            # out = in/32 + bq (Identity shares exp's table)
                            nc.scalar.activation(dst, src, IDENT, bias=bqs,
                                                 scale=INV32)
                        else:
                            nc.vector.tensor_scalar(
                                dst, src, INV32, bqs,
                                mybir.AluOpType.mult, mybir.AluOpType.add)
                    else:
                        if hp == 0:
                            nc.scalar.mul(dst, src, INV32)
                        else:
                            nc.vector.tensor_scalar_mul(dst, src, INV32)

            def emit_v(tt, ts):
                pv = ps.tile([128, 256], F32, tag="pq", name=f"pv_{tt}_{ts}")
                tb = slice((4 * tt + ts) * 128, (4 * tt + ts + 1) * 128)
                terms = [(xth, wvh), (xth, wvl), (xtl, wvh)]
                for ti, (xx, w) in enumerate(terms):
                    for j in range(4):
                        nc.tensor.matmul(
                            pv[:], xx[:, 2 * j:2 * j + 2, tb], w[:, j],
                            start=(ti == 0 and j == 0),
                            stop=(ti == 2 and j == 3), perf_mode=DR)
                nc.vector.tensor_scalar_mul(
                    vp[:, :, 4 * tt + ts, 0:64],
                    pv[:].rearrange("p (h e) -> p h e", e=HD), INV32)

            def emit_scores(a, qi, kt):
                """fp8 DoubleRow QK^T for head pair a at (qi, kt), one exp
                over both heads, diagonal mask multiply. Returns (ptp, c_lo).
                """
                rr = kt - 4 * qi
                c_lo = 0 if rr < 0 else 128 * rr
                s = ps.tile([128, 2, TQ], F32, tag="s", name=f"s_{a}_{qi}_{kt}")
                for i in range(2):
                    nc.tensor.matmul(
                        s[:, i, c_lo:],
                        qkt[32 * i:32 * i + 32, a, :, 1,
                            kt * TK:(kt + 1) * TK],
                        qkt[32 * i:32 * i + 32, a, :, 0,
                            qi * TQ + c_lo:(qi + 1) * TQ],
                        start=True, stop=True, perf_mode=DR)
                ptp = ptp_p.tile([128, 2, TQ], BF16, tag="pt",
                                 name=f"pt_{a}_{qi}_{kt}")
                nc.scalar.activation(ptp[:, :, c_lo:], s[:, :, c_lo:], EXP)
                if rr >= 0:
                    c0 = (3 - rr) * 128
                    nc.vector.tensor_tensor(
                        ptp[:, :, c_lo:c_lo + 128], ptp[:, :, c_lo:c_lo + 128],
                        mask[:, :, c0 + c_lo:c0 + c_lo + 128], MULT)
                return ptp, c_lo

            def emit_c(a, qi, fillers, rem_iters, pre=None, prefetch=None):
                """rem_iters: kt iterations left in this tt including this
                pair's — paces the filler drain across both pairs. `pre` is
                this pair's already-prefetched kt=0 scores; `prefetch` emits
                the NEXT pair's kt=0 scores during our last iteration so the
                pair boundary never exposes a full exp latency. Returns the
                prefetched scores for the next pair."""
                av = [ps_av.tile([65, TQ], F32, tag=f"av{i}",
                                 name=f"av{i}_{a}_{qi}") for i in range(2)]
                nkt = 4 * qi + 4
                pts, c_lo = pre if pre is not None else emit_scores(a, qi, 0)
                nxt_pre = None
                for kt in range(nkt):
                    if kt + 1 < nkt:
                        nxt = emit_scores(a, qi, kt + 1)
                    else:
                        nxt = (None, 0)
                        if prefetch is not None:
                            nxt_pre = prefetch()
                    n_pop = min(len(fillers),
                                -(-len(fillers) // max(1, rem_iters)))
                    rem_iters -= 1
                    for _ in range(n_pop):
                        fillers.popleft()()
                    for i in range(2):
                        h = 2 * a + i
                        nc.tensor.matmul(
                            av[i][0:65, c_lo:], vp[:, h, kt, :],
                            pts[:, i, c_lo:],
                            start=(kt == 0), stop=(kt == nkt - 1),
                            skip_group_check=True)
                    pts, c_lo = nxt
                # Evacuate av to SBUF immediately so the PSUM banks free for
                # the next pair; normalize off the critical path from SBUF.
                # Rows 0:64 are O^T, row 64 the softmax denominator. The last
                # block's banks are never reused — normalize from PSUM
                # directly to shorten the tail chain.
                last = qi == NQT - 1
                if last:
                    avs = av
                else:
                    avs = []
                    for i in range(2):
                        t = nrm_p.tile([65, TQ], F32, tag=f"avs{i}",
                                       name=f"avs{i}_{a}_{qi}")
                        nc.vector.tensor_copy(t[:], av[i][0:65, :])
                        avs.append(t)
                bcs = []
                for i in range(2):
                    rec = nrm_p.tile([1, TQ], F32, tag="rec",
                                     name=f"rec_{a}_{qi}_{i}")
                    nc.vector.reciprocal(rec[:], avs[i][64:65, :])
                    bc = nrm_p.tile([64, TQ], F32, tag="bc",
                                    name=f"bc_{a}_{qi}_{i}")
                    nc.gpsimd.partition_broadcast(bc[:], rec[:])
                    bcs.append(bc)
                if last:
                    # chunk per 128-token slice on DVE so the final
                    # out-projection units can start per-slice
                    for cs in range(4):
                        for i in range(2):
                            sl = slice(cs * 128, (cs + 1) * 128)
                            nc.vector.tensor_tensor(
                                ot[64 * i:64 * i + 64, a,
                                   qi * TQ + cs * 128:qi * TQ + (cs + 1) * 128],
                                avs[i][0:64, sl], bcs[i][:, sl], MULT)
                else:
                    for i in range(2):
                        nc.vector.tensor_tensor(
                            ot[64 * i:64 * i + 64, a,
                               qi * TQ:(qi + 1) * TQ],
                            avs[i][0:64, :], bcs[i][:], MULT)
                return nxt_pre

            def po_fillers(qi):
                def emit_po(ts, dt, ob):
                    po = ps.tile([128, TQ], F32, tag="pq",
                                 name=f"po_{ts}_{dt}")
                    for ft in range(2):
                        nc.tensor.matmul(
                            po[:], ot[:, ft, ts * 128:(ts + 1) * 128],
                            wout[:, ft, dt * 512:(dt + 1) * 512],
                            start=(ft == 0), stop=(ft == 1))
                    nc.vector.tensor_copy(ob[:, dt * 512:(dt + 1) * 512],
                                          po[:])
                    if dt == 1:
                        # one DMA per 128-token row (HWDGE overhead halves)
                        nc.sync.dma_start(out_ap[ts * 128:(ts + 1) * 128, :],
                                          ob[:])
                obs = {}
                def unit(ts, dt):
                    if dt == 0:
                        obs[ts] = ob_p.tile([128, 2 * TQ], BF16, tag="ob",
                                            name=f"ob_{ts}")
                    emit_po(ts, dt, obs[ts])
                return [
                    (lambda ts=ts, dt=dt: unit(ts, dt))
                    for ts in range(4 * qi, 4 * qi + 4) for dt in range(2)
                ]

            def qkv_fillers(tt):
                return ([(lambda fb=fb: emit_fb(tt, fb)) for fb in range(4)]
                        + [(lambda ts=ts: emit_v(tt, ts)) for ts in range(4)])

            # Block 0's Q/K projections run up front; everything else (V(0),
            # later blocks' QKV, all deferrable out-projections) slots into
            # emit_c's filler gaps so the PE never idles on the exp -> AV
            # dependency. The out-projections all land in the last block,
            # whose long kt loops otherwise run out of filler work.
            for fb in range(4):
                emit_fb(0, fb)
            pre = None
            for tt in range(NQT):
                fl = deque()
                if tt == 0:
                    fl.extend([(lambda ts=ts: emit_v(0, ts))
                               for ts in range(4)])
                if tt + 1 < NQT:
                    fl.extend(qkv_fillers(tt + 1))
                else:
                    for qi in range(NQT - 1):
                        fl.extend(po_fillers(qi))
                pre = emit_c(0, tt, fl, 2 * (4 * tt + 4), pre,
                             prefetch=lambda tt=tt: emit_scores(1, tt, 0))
                nxt_pf = ((lambda tt=tt: emit_scores(0, tt + 1, 0))
                          if tt + 1 < NQT else None)
                pre = emit_c(1, tt, fl, 4 * tt + 4, pre, prefetch=nxt_pf)
                while fl:
                    fl.popleft()()
            for f in po_fillers(NQT - 1):
                f()


_CACHE = {}


def _program():
    if "nc" in _CACHE:
        return _CACHE["nc"]
    nc = bacc.Bacc("TRN2", target_bir_lowering=False, debug=False)
    ins = {
        "xth": nc.dram_tensor("xth", [128, NDT, T], F8,
                              kind="ExternalInput").ap(),
        "xtl": nc.dram_tensor("xtl", [128, NDT, T], F8,
                              kind="ExternalInput").ap(),
        "wqkh": nc.dram_tensor("wqkh", [128, 4, 2, 512], F8,
                               kind="ExternalInput").ap(),
        "wqkl": nc.dram_tensor("wqkl", [128, 4, 2, 512], F8,
                               kind="ExternalInput").ap(),
        "bq": nc.dram_tensor("bq", [128, 2], F32, kind="ExternalInput").ap(),
        "wvh": nc.dram_tensor("wvh", [128, 4, 2, 256], F8,
                              kind="ExternalInput").ap(),
        "wvl": nc.dram_tensor("wvl", [128, 4, 2, 256], F8,
                              kind="ExternalInput").ap(),
        "wout": nc.dram_tensor("wout", [128, 2, 1024], BF16,
                               kind="ExternalInput").ap(),
        "mask": nc.dram_tensor("mask", [128, 2, 896], BF16,
                               kind="ExternalInput").ap(),
    }
    out = nc.dram_tensor("out", [T, D], BF16, kind="ExternalOutput").ap()
    with tile.TileContext(nc) as tc:
        _build_mha(tc, out, ins)
    nc.compile()
    _CACHE["nc"] = nc
    return nc


def _in_maps(x, Wqkv, bqkv, Wout):
    x = np.asarray(x, dtype=np.float32)
    Wqkv = np.asarray(Wqkv, dtype=np.float32)
    bqkv = np.asarray(bqkv, dtype=np.float32)
    Wout = np.asarray(Wout, dtype=np.float32)
    scale = np.float32(1.0 / np.sqrt(HD))
    maskbase = (np.arange(128)[:, None] <= np.arange(896)[None, :] - 384)
    mask = np.ascontiguousarray(
        np.broadcast_to(maskbase[:, None, :], (128, 2, 896))).astype(BFNP)
    maps = []
    for c in range(NCORES):
        b, hg = c // 4, c % 4
        hs = [4 * hg + i for i in range(HL)]
        # [1024, 256] per-projection slices for this head group
        q_cols = np.concatenate(
            [Wqkv[:, h * HD:(h + 1) * HD] for h in hs], axis=1) * scale
        k_cols = np.concatenate(
            [Wqkv[:, D + h * HD:D + (h + 1) * HD] for h in hs], axis=1)
        v_cols = np.concatenate(
            [Wqkv[:, 2 * D + h * HD:2 * D + (h + 1) * HD] for h in hs], axis=1)
        # A/B half split: [1024, 4 heads, 2 halves, 32] -> QA|QB / KA|KB
        qr = q_cols.reshape(D, HL, 2, 32)
        kr = k_cols.reshape(D, HL, 2, 32)
        wqk_full = np.concatenate(
            [qr[:, :, 0].reshape(D, 128), qr[:, :, 1].reshape(D, 128),
             kr[:, :, 0].reshape(D, 128), kr[:, :, 1].reshape(D, 128)],
            axis=1)                                    # [1024, 512]

        def w_split(w):
            """x32 scale (keeps the lo residual in fp8 normal range), then
            fp8 hi/lo split in the [128, 4, 2, C] DoubleRow layout."""
            w = w * np.float32(32.0)
            C = w.shape[1]
            hi = w.astype(F8NP)
            lo = (w - hi.astype(np.float32)).astype(F8NP)
            def lay(a):
                return np.ascontiguousarray(
                    a.reshape(4, 2, 128, C).transpose(2, 0, 1, 3))
            return lay(hi), lay(lo)

        wqkh, wqkl = w_split(wqk_full)
        wvh, wvl = w_split(v_cols)
        bq_cols = np.concatenate(
            [bqkv[h * HD:(h + 1) * HD] for h in hs]) * scale
        bqr = bq_cols.reshape(HL, 2, 32)
        bq = np.ascontiguousarray(
            np.stack([bqr[:, 0].reshape(128), bqr[:, 1].reshape(128)],
                     axis=1)).astype(np.float32)       # [128, 2]
        wo = np.concatenate([Wout[h * HD:(h + 1) * HD, :] for h in hs], axis=0)
        wo = np.ascontiguousarray(
            wo.reshape(2, 128, D).transpose(1, 0, 2)).astype(BFNP)
        xtf = np.ascontiguousarray(
            x[b].T.reshape(NDT, 128, T).transpose(1, 0, 2))
        xth = xtf.astype(F8NP)
        xtl = (xtf - xth.astype(np.float32)).astype(F8NP)
        maps.append({
            "xth": xth,
            "xtl": xtl,
            "wqkh": wqkh,
            "wqkl": wqkl,
            "bq": bq,
            "wvh": wvh,
            "wvl": wvl,
            "wout": wo,
            "mask": mask,
        })
    return maps


def kernel(x, Wqkv, bqkv, Wout, bout):
    global LAST_RESULTS
    nc = _program()
    maps = _in_maps(x, Wqkv, bqkv, Wout)
    res = run_bass_kernel_spmd(nc, maps, list(range(NCORES)))
    LAST_RESULTS = res
    bqkv = np.asarray(bqkv, dtype=np.float32)
    bout = np.asarray(bout, dtype=np.float32)
    # V bias folded: softmax weights sum to 1, so out += bv @ Wout exactly.
    bout_folded = bout + np.asarray(Wout, np.float32).T @ bqkv[2 * D:]
    out = np.empty((B, T, D), dtype=np.float32)
    for b in range(B):
        acc = np.asarray(res.results[4 * b]["out"], np.float32)
        for hg in range(1, 4):
            acc = acc + np.asarray(res.results[4 * b + hg]["out"], np.float32)
        out[b] = acc + bout_folded[None, :]
    return out
